# revision 59
# baseline (speedup 1.0000x reference)
"""TRN2 Bass kernel for nn_HCSMoEQwen3MoeSparseMoeBlock (8-core).

Host computes the router (fp32 numpy, matching reference softmax/top-8
semantics) and the per-(token, group) combined weight w_tg.  Only
(token, group) pairs with w > 0 are computed on device (~64% density).

Schedule: each group's active tokens are carved into single-group parts
of sizes SEGT = (368, 480, 512) — 8 parts of each size globally, found
by a small DP — so every core runs an identical 3-segment, 1360-token
program (vs 2048 dense).  Per-core inputs: gathered x (bf16,
partition-major so DMA reads are 8-16KB bursts), one gu/dn weight slot
per segment (bf16, gate/up column-interleaved), per-token weights.
Host scatter-adds the bf16 partial outputs (unique tokens per segment).

Device per segment:
  M1 transposed: hT[o-tile, t] = guT-tile-stationary.T @ xT (bf16, fp32
  PSUM, full PE rate, no transposes); gate/up PSUM bank pair -> Silu on
  Scalar * up on DVE -> actT [i, t] bf16.
  M2: y[t, hb] = actT-subtile-stationary.T @ dnT (bf16); two hb blocks
  accumulate into one 2-bank PSUM tile so a single wide DVE scale-by-w
  + one DMA drains the pair (drain keeps up with PE) -> bf16 DRAM.
Software pipelining: M1'(s+1)'s first two j-pairs are emitted before
M2(s) so DVE mults queue ahead of y-scales; gate/up (and M2 half-pair)
matmuls interleave banks every instruction so run boundaries stay
hidden; steady-state input prefetch rides M2's otherwise-idle DMA
windows instead of contending with M1' SBUF reads; head interleaves
x/gu k-quarters so the first matmul starts ~11.5us in (vs ~9us fixed
pre-DMA startup).
"""
import numpy as np
import ml_dtypes

import concourse.bass as bass
import concourse.mybir as mybir
import concourse.tile as tile
from concourse import bacc
from concourse.bass_utils import run_bass_kernel_spmd

BF = ml_dtypes.bfloat16

T = 2048
H = 2048
I2 = 1536
I = 768
E = 32
G = 8
TOP_K = 8
KO = H // 128          # 16 k-tiles
JO = I // 128          # 6 i-tiles
NSEG = 3
# per-core segment token counts (identical on every core; 8 parts of each
# size exist globally, single-group each).  Sizes need not be multiples of
# 128: M1' cost scales with tokens, M2 with ceil(Ts/128), so (368,480,512)
# = 1360 tokens/core beats (384,512,512) = 1408 at equal M2 cost.
SEGT = (368, 480, 512)
SEGSUB = tuple(-(-t // 128) for t in SEGT)   # M2 sub-chunks: (3, 4, 4)
CHOFF = (0, 3, 7)      # w-column offset per segment
TOFF = (0, 368, 848)   # token slot offset per segment
NCH = sum(SEGSUB)      # 11 w columns per core
NT = sum(SEGT)         # 1360 token slots per core
HBW = 512

F32 = mybir.dt.float32
BF16 = mybir.dt.bfloat16
AX = mybir.AxisListType.X
OP = mybir.AluOpType
ACTF = mybir.ActivationFunctionType

_CACHED_NC = None

# gate j-tile at cols [256j, 256j+128), up j-tile at [256j+128, 256j+256)
_GU_PERM = np.concatenate(
    [np.r_[128 * j:128 * j + 128, I + 128 * j:I + 128 * j + 128]
     for j in range(JO)]
)


def _build():
    global _CACHED_NC
    if _CACHED_NC is not None:
        return _CACHED_NC
    nc = bacc.Bacc("TRN2", target_bir_lowering=False, debug=False,
                   num_devices=G)

    # partition-major inputs: per-partition rows are contiguous in DRAM,
    # so DMA reads come in 8-24KB bursts instead of 0.5-1KB lines.
    x_ds = [nc.dram_tensor(f"x{s}", [128, KO * SEGT[s]], BF16,
                           kind="ExternalInput") for s in range(NSEG)]
    gu_ds = [nc.dram_tensor(f"gu{s}", [128, JO * KO * 256], BF16,
                            kind="ExternalInput") for s in range(NSEG)]
    dn_ds = [nc.dram_tensor(f"dn{s}", [128, JO * H], BF16,
                            kind="ExternalInput") for s in range(NSEG)]
    w_d = nc.dram_tensor("w", [128, NCH], F32, kind="ExternalInput")
    y_d = nc.dram_tensor("y", [NT, H], BF16, kind="ExternalOutput")

    x_aps = [x_ds[s].ap().rearrange("p (k t) -> p k t", k=KO)
             for s in range(NSEG)]
    gu_aps = [gu_ds[s].ap().rearrange("p (j k c) -> p j k c", j=JO, k=KO)
              for s in range(NSEG)]
    dn_aps = [dn_ds[s].ap().rearrange("p (j h) -> p j h", j=JO)
              for s in range(NSEG)]

    with tile.TileContext(nc) as tc:
        with (
            tc.tile_pool(name="const", bufs=1) as cpool,
            tc.tile_pool(name="guw", bufs=2) as gupool,
            tc.tile_pool(name="dnw", bufs=2) as dnpool,
            tc.tile_pool(name="xin", bufs=2) as xpool,
            tc.tile_pool(name="acts", bufs=2) as apool,
            tc.tile_pool(name="silu", bufs=2) as spool,
            tc.tile_pool(name="yout", bufs=4) as ypool,
            tc.tile_pool(name="ph", bufs=2, space="PSUM") as pph,
            tc.tile_pool(name="py", bufs=2, space="PSUM") as ppy,
        ):
            x_tiles = {}
            gu_tiles = {}
            dn_tiles = {}

            def load_x(s, halves=False):
                t = xpool.tile([128, KO, 512], BF16, tag="x", name=f"x{s}")
                if halves:
                    hk = KO // 2
                    nc.sync.dma_start(t[:, :hk, :SEGT[s]], x_aps[s][:, :hk])
                    nc.sync.dma_start(t[:, hk:, :SEGT[s]], x_aps[s][:, hk:])
                else:
                    nc.sync.dma_start(t[:, :, :SEGT[s]], x_aps[s])
                x_tiles[s] = t

            def load_gu(s, pieces):
                if s not in gu_tiles:
                    gu_tiles[s] = gupool.tile([128, KO, I2], BF16, tag="gu",
                                              name=f"gu{s}")
                t = gu_tiles[s]
                for pc in pieces:
                    nc.sync.dma_start(
                        t[:, :, 256 * pc:256 * (pc + 1)],
                        gu_aps[s][:, pc],
                    )

            def load_dn(s):
                t = dnpool.tile([128, JO, H], BF16, tag="dn", name=f"dn{s}")
                nc.sync.dma_start(t[:], dn_aps[s])
                dn_tiles[s] = t

            # head: x(0) in k-halves, first gu piece split gate/up so the
            # very first accumulation run can start as early as possible
            # head FIFO in first-need order: x/gu k-halves for the very
            # first accumulation run, then the rest
            xt0 = xpool.tile([128, KO, 512], BF16, tag="x", name="x0")
            x_tiles[0] = xt0
            gt0 = gupool.tile([128, KO, I2], BF16, tag="gu", name="gu0")
            gu_tiles[0] = gt0
            Ts0 = SEGT[0]
            # interleave x / gu-gate k-quarters so the j0 gate run can
            # begin after ~0.5MB has landed
            for q in range(4):
                ka, kb = 4 * q, 4 * (q + 1)
                nc.sync.dma_start(xt0[:, ka:kb, :Ts0], x_aps[0][:, ka:kb])
                nc.sync.dma_start(gt0[:, ka:kb, 0:256],
                                  gu_aps[0][:, 0, ka:kb, :])
            load_gu(0, (1, 2, 3, 4, 5))
            w_sb = cpool.tile([128, NCH], F32, tag="w")
            nc.sync.dma_start(w_sb[:], w_d.ap())
            load_dn(0)

            actT_tiles = {}

            def m1(s, jrange):
                Ts = SEGT[s]
                xc = x_tiles[s]
                guc = gu_tiles[s]
                if s not in actT_tiles:
                    actT_tiles[s] = apool.tile([128, JO, 512], BF16,
                                               tag="act", name=f"act{s}")
                actT = actT_tiles[s]
                for j in jrange:
                    a_ps = pph.tile([128, 512], F32, tag="hg",
                                    name=f"hg{s}_{j}")
                    b_ps = pph.tile([128, 512], F32, tag="hu",
                                    name=f"hu{s}_{j}")
                    # gate/up interleaved: banks alternate every instruction
                    # so run boundaries and stationary loads stay hidden
                    for k in range(KO):
                        nc.tensor.matmul(
                            a_ps[:, :Ts], guc[:, k, 256 * j:256 * j + 128],
                            xc[:, k, :Ts],
                            start=(k == 0), stop=(k == KO - 1),
                        )
                        nc.tensor.matmul(
                            b_ps[:, :Ts],
                            guc[:, k, 256 * j + 128:256 * j + 256],
                            xc[:, k, :Ts],
                            start=(k == 0), stop=(k == KO - 1),
                        )
                    sl = spool.tile([128, 512], F32, tag="sl",
                                    name=f"sl{s}_{j}")
                    nc.scalar.activation(sl[:, :Ts], a_ps[:, :Ts], ACTF.Silu)
                    nc.vector.tensor_tensor(actT[:, j, :Ts], sl[:, :Ts],
                                            b_ps[:, :Ts], OP.mult)
                    # chain start only: seg1's lookahead inputs must load
                    # during M1'(0); all later prefetches ride M2 windows
                    if s == 0:
                        if j == 0:
                            load_x(1)
                        elif j == 1:
                            load_gu(1, (0, 1, 2))


            def m2(s):
                Ts = SEGT[s]
                x_tiles.pop(s)
                gu_tiles.pop(s)
                actT = actT_tiles.pop(s)
                dnc = dn_tiles.pop(s)
                for sub in range(SEGSUB[s]):
                    np_ = min(128, Ts - 128 * sub)
                    # input prefetch rides M2's otherwise-idle DMA window
                    # instead of contending with M1' SBUF reads
                    if sub == 0 and s + 1 < NSEG:
                        load_gu(s + 1, (3, 4, 5))
                    elif sub == 1 and s + 1 < NSEG:
                        load_dn(s + 1)
                    elif sub == 2 and s + 2 < NSEG:
                        load_x(s + 2)
                        load_gu(s + 2, (0, 1, 2))
                    # two hb blocks share one 2-bank PSUM tile: a single
                    # wide DVE scale + DMA per pair halves drain overhead
                    for hp in range(H // (2 * HBW)):
                        y_ps = ppy.tile([128, 2 * HBW], F32, tag="y_ps",
                                        name=f"y{s}_{sub}_{hp}")
                        # j outer, halves inner: consecutive matmuls share
                        # the stationary and alternate banks
                        for j in range(JO):
                            for hh in range(2):
                                hb = 2 * hp + hh
                                nc.tensor.matmul(
                                    y_ps[:np_, HBW * hh:HBW * (hh + 1)],
                                    actT[:, j, 128 * sub:128 * sub + np_],
                                    dnc[:, j, HBW * hb:HBW * (hb + 1)],
                                    start=(j == 0), stop=(j == JO - 1),
                                )
                        y_sb = ypool.tile([128, 2 * HBW], BF16, tag="y_sb",
                                          name=f"ys{s}_{sub}_{hp}")
                        ci = CHOFF[s] + sub
                        nc.vector.tensor_scalar(y_sb[:np_], y_ps[:np_],
                                                w_sb[:np_, ci:ci + 1], None,
                                                OP.mult)
                        # y stores ride the Activation hwdge queue so
                        # outputs don't share the input queue's bandwidth
                        nc.scalar.dma_start(
                            y_d.ap()[TOFF[s] + 128 * sub:
                                     TOFF[s] + 128 * sub + np_,
                                     2 * HBW * hp:2 * HBW * (hp + 1)],
                            y_sb[:np_],
                        )

            # 2-stage lookahead: M1'(s+1)'s first two j-pairs are emitted
            # before M2(s), so their DVE mults queue ahead of M2(s)'s
            # y-scales and the seg boundary never stalls on the DVE FIFO.
            m1(0, range(0, 2))
            for s in range(NSEG):
                m1(s, range(2, JO))
                if s + 1 < NSEG:
                    m1(s + 1, range(0, 2))
                m2(s)
    nc.compile()
    _CACHED_NC = nc
    return nc


def _route(x32, gw32, mg):
    """fp32 router matching reference: softmax, top-8, renorm, per-group sum."""
    logits = x32 @ gw32.T
    m = logits.max(axis=-1, keepdims=True)
    p = np.exp(logits - m)
    p /= p.sum(axis=-1, keepdims=True)
    idx = np.argsort(-p, axis=-1, kind="stable")[:, :TOP_K]
    val = np.take_along_axis(p, idx, axis=-1)
    val = val / val.sum(axis=-1, keepdims=True)
    sel = mg[idx]  # [T, K] group ids
    w_tg = np.zeros((x32.shape[0], G), dtype=np.float32)
    np.add.at(w_tg, (np.arange(x32.shape[0])[:, None], sel), val)
    return w_tg


def _solve_parts(counts):
    """Assign each group a multiset of parts from SEGT (8 of each size
    available) covering its token count; all 24 parts are consumed
    (leftovers become all-pad parts on group 0).  Returns per-group
    (n_of_size0, n_of_size1, n_of_size2), minimizing total padding."""
    sizes = SEGT
    states = {(0, 0, 0): (0, ())}
    for c in counts:
        new = {}
        for used, (pad, plan) in states.items():
            for m0 in range(0, 8 - used[0] + 1):
                for m1 in range(0, 8 - used[1] + 1):
                    rem = c - m0 * sizes[0] - m1 * sizes[1]
                    m2min = max(0, -(-rem // sizes[2]))
                    for m2 in (m2min, m2min + 1):
                        if used[2] + m2 > 8:
                            continue
                        cov = m0 * sizes[0] + m1 * sizes[1] + m2 * sizes[2]
                        if cov < c:
                            continue
                        key = (used[0] + m0, used[1] + m1, used[2] + m2)
                        cand = (pad + cov - c, plan + ((m0, m1, m2),))
                        if key not in new or cand[0] < new[key][0]:
                            new[key] = cand
        states = new
    best = None
    for used, (pad, plan) in states.items():
        # leftovers are pure padding
        extra = sum((8 - u) * s for u, s in zip(used, sizes))
        if best is None or pad + extra < best[0]:
            left = tuple(8 - u for u in used)
            best = (pad + extra, plan, left)
    if best is None:
        raise RuntimeError(f"no schedule for token counts {counts}")
    plan = list(best[1])
    plan[0] = tuple(p + l for p, l in zip(plan[0], best[2]))
    return plan


def _schedule(w_tg):
    """Build 8 parts of each SEGT size: (group, token_slots) lists."""
    toks = [np.nonzero(w_tg[:, g] > 0.0)[0] for g in range(G)]
    plan = _solve_parts([len(t) for t in toks])
    parts = {0: [], 1: [], 2: []}
    for g, mult in enumerate(plan):
        cap = sum(m * s for m, s in zip(mult, SEGT))
        slots = np.full(cap, -1, dtype=np.int64)
        slots[:len(toks[g])] = toks[g]
        off = 0
        for si in range(NSEG):
            for _ in range(mult[si]):
                parts[si].append((g, slots[off:off + SEGT[si]]))
                off += SEGT[si]
    return parts


def prepare_in_maps(hidden_states, gate_weight, gate_up_proj, down_proj,
                    merge_groups, dominant_experts):
    x32 = np.asarray(hidden_states, dtype=np.float32).reshape(T, H)
    gw32 = np.asarray(gate_weight, dtype=np.float32)
    mg = np.asarray(merge_groups).astype(np.int64)
    de = np.asarray(dominant_experts).astype(np.int64)
    gup = np.asarray(gate_up_proj, dtype=np.float32)
    dnp_ = np.asarray(down_proj, dtype=np.float32)

    w_tg = _route(x32, gw32, mg)
    parts = _schedule(w_tg)

    # per-expert transformed weights (bf16, partition-major), cached
    guT_c, dnT_c = {}, {}
    for g in range(G):
        e = int(de[g])
        if e not in guT_c:
            guT = gup[e].T[:, _GU_PERM].astype(BF)           # [H, 2I]
            # [p, j, k, c]: element (k*128+p, 256j+c)
            guT_c[e] = np.ascontiguousarray(
                guT.reshape(KO, 128, JO, 256).transpose(1, 2, 0, 3)
            ).reshape(128, -1)
            dnT = dnp_[e].T.astype(BF)                       # [I, H]
            dnT_c[e] = np.ascontiguousarray(
                dnT.reshape(JO, 128, H).transpose(1, 0, 2)
            ).reshape(128, -1)                               # [p, j, h]

    x_bf = x32.astype(BF)
    in_maps = []
    slot_lists = []
    for c in range(G):
        segs = [parts[0][c], parts[1][c], parts[2][c]]
        slots = np.concatenate([sg[1] for sg in segs])       # [NT], -1 pads
        slot_lists.append(slots)
        # w columns follow the (seg, sub) grid of 128-token sub-chunks
        w_dev = np.zeros((128, NCH), dtype=np.float32)
        for s, (g, sl) in enumerate(segs):
            valid = sl >= 0
            wseg = np.zeros(len(sl), dtype=np.float32)
            wseg[valid] = w_tg[sl[valid], g]
            for sub in range(SEGSUB[s]):
                col = wseg[128 * sub:128 * (sub + 1)]
                w_dev[:len(col), CHOFF[s] + sub] = col
        in_map = {"w": np.ascontiguousarray(w_dev)}
        for s, (g, sl) in enumerate(segs):
            cl = np.where(sl < 0, 0, sl)
            xs = x_bf[cl]                                    # [Ts, H]
            # [p, k, t]: element (t, k*128+p)
            in_map[f"x{s}"] = np.ascontiguousarray(
                xs.reshape(len(sl), KO, 128).transpose(2, 1, 0)
            ).reshape(128, -1)
            e = int(de[g])
            in_map[f"gu{s}"] = guT_c[e]
            in_map[f"dn{s}"] = dnT_c[e]
        in_maps.append(in_map)
    return in_maps, slot_lists


def kernel(hidden_states, gate_weight, gate_up_proj, down_proj,
           merge_groups, dominant_experts):
    in_maps, slot_lists = prepare_in_maps(
        hidden_states, gate_weight, gate_up_proj, down_proj,
        merge_groups, dominant_experts)
    nc = _build()
    res = run_bass_kernel_spmd(nc, in_maps, core_ids=list(range(G)),
                               trace=False)
    out = np.zeros((T, H), dtype=np.float64)
    for c, r in enumerate(res.results):
        y = np.asarray(r["y"], dtype=np.float64)             # [NT, H]
        slots = slot_lists[c]
        # segments have unique tokens internally; add per segment
        for so, n in zip(TOFF, SEGT):
            sl = slots[so:so + n]
            valid = sl >= 0
            out[sl[valid]] += y[so:so + n][valid]
    return out.astype(np.float32).reshape(1, T, H)


# revision 60
# speedup vs baseline: 1.0083x; 1.0083x over previous
"""TRN2 Bass kernel for nn_HCSMoEQwen3MoeSparseMoeBlock (8-core).

Host computes the router (fp32 numpy, matching reference softmax/top-8
semantics) and the per-(token, group) combined weight w_tg.  Only
(token, group) pairs with w > 0 are computed on device (~64% density).

Schedule: each group's active tokens are carved into single-group parts
of sizes SEGT = (368, 480, 512) — 8 parts of each size globally, found
by a small DP — so every core runs an identical 3-segment, 1360-token
program (vs 2048 dense).  Per-core inputs: gathered x (bf16,
partition-major so DMA reads are 8-16KB bursts), one gu/dn weight slot
per segment (bf16, gate/up column-interleaved), per-token weights.
Host scatter-adds the bf16 partial outputs (unique tokens per segment).

Device per segment:
  M1 transposed: hT[o-tile, t] = guT-tile-stationary.T @ xT (bf16, fp32
  PSUM, full PE rate, no transposes); gate/up PSUM bank pair -> Silu on
  Scalar * up on DVE -> actT [i, t] bf16.
  M2: y[t, hb] = actT-subtile-stationary.T @ dnT (bf16); two hb blocks
  accumulate into one 2-bank PSUM tile so a single wide DVE scale-by-w
  + one DMA drains the pair (drain keeps up with PE) -> bf16 DRAM.
Software pipelining: M1'(s+1)'s first two j-pairs are emitted before
M2(s) so DVE mults queue ahead of y-scales; gate/up (and M2 half-pair)
matmuls interleave banks every instruction so run boundaries stay
hidden; steady-state input prefetch rides M2's otherwise-idle DMA
windows instead of contending with M1' SBUF reads; head interleaves
x/gu k-quarters so the first matmul starts ~11.5us in (vs ~9us fixed
pre-DMA startup).
"""
import numpy as np
import ml_dtypes

import concourse.bass as bass
import concourse.mybir as mybir
import concourse.tile as tile
from concourse import bacc
from concourse.bass_utils import run_bass_kernel_spmd

BF = ml_dtypes.bfloat16

T = 2048
H = 2048
I2 = 1536
I = 768
E = 32
G = 8
TOP_K = 8
KO = H // 128          # 16 k-tiles
JO = I // 128          # 6 i-tiles
NSEG = 3
# per-core segment token counts (identical on every core; 8 parts of each
# size exist globally, single-group each).  Sizes need not be multiples of
# 128: M1' cost scales with tokens, M2 with ceil(Ts/128), so (368,480,512)
# = 1360 tokens/core beats (384,512,512) = 1408 at equal M2 cost.
SEGT = (368, 480, 512)
SEGSUB = tuple(-(-t // 128) for t in SEGT)   # M2 sub-chunks: (3, 4, 4)
CHOFF = (0, 3, 7)      # w-column offset per segment
TOFF = (0, 368, 848)   # token slot offset per segment
NCH = sum(SEGSUB)      # 11 w columns per core
NT = sum(SEGT)         # 1360 token slots per core
HBW = 512

F32 = mybir.dt.float32
BF16 = mybir.dt.bfloat16
AX = mybir.AxisListType.X
OP = mybir.AluOpType
ACTF = mybir.ActivationFunctionType

_CACHED_NC = None

# gate j-tile at cols [256j, 256j+128), up j-tile at [256j+128, 256j+256)
_GU_PERM = np.concatenate(
    [np.r_[128 * j:128 * j + 128, I + 128 * j:I + 128 * j + 128]
     for j in range(JO)]
)


def _build():
    global _CACHED_NC
    if _CACHED_NC is not None:
        return _CACHED_NC
    nc = bacc.Bacc("TRN2", target_bir_lowering=False, debug=False,
                   num_devices=G)

    # partition-major inputs: per-partition rows are contiguous in DRAM,
    # so DMA reads come in 8-24KB bursts instead of 0.5-1KB lines.
    x_ds = [nc.dram_tensor(f"x{s}", [128, KO * SEGT[s]], BF16,
                           kind="ExternalInput") for s in range(NSEG)]
    gu_ds = [nc.dram_tensor(f"gu{s}", [128, JO * KO * 256], BF16,
                            kind="ExternalInput") for s in range(NSEG)]
    dn_ds = [nc.dram_tensor(f"dn{s}", [128, JO * H], BF16,
                            kind="ExternalInput") for s in range(NSEG)]
    w_d = nc.dram_tensor("w", [128, NCH], F32, kind="ExternalInput")
    y_d = nc.dram_tensor("y", [NT, H], BF16, kind="ExternalOutput")

    x_aps = [x_ds[s].ap().rearrange("p (k t) -> p k t", k=KO)
             for s in range(NSEG)]
    gu_aps = [gu_ds[s].ap().rearrange("p (j k c) -> p j k c", j=JO, k=KO)
              for s in range(NSEG)]
    dn_aps = [dn_ds[s].ap().rearrange("p (j h) -> p j h", j=JO)
              for s in range(NSEG)]

    with tile.TileContext(nc) as tc:
        with (
            tc.tile_pool(name="const", bufs=1) as cpool,
            tc.tile_pool(name="guw", bufs=2) as gupool,
            tc.tile_pool(name="dnw", bufs=2) as dnpool,
            tc.tile_pool(name="xin", bufs=2) as xpool,
            tc.tile_pool(name="acts", bufs=2) as apool,
            tc.tile_pool(name="silu", bufs=2) as spool,
            tc.tile_pool(name="yout", bufs=4) as ypool,
            tc.tile_pool(name="ph", bufs=2, space="PSUM") as pph,
            tc.tile_pool(name="py", bufs=2, space="PSUM") as ppy,
        ):
            x_tiles = {}
            gu_tiles = {}
            dn_tiles = {}

            def load_x(s, halves=False):
                t = xpool.tile([128, KO, 512], BF16, tag="x", name=f"x{s}")
                if halves:
                    hk = KO // 2
                    nc.sync.dma_start(t[:, :hk, :SEGT[s]], x_aps[s][:, :hk])
                    nc.sync.dma_start(t[:, hk:, :SEGT[s]], x_aps[s][:, hk:])
                else:
                    nc.sync.dma_start(t[:, :, :SEGT[s]], x_aps[s])
                x_tiles[s] = t

            def load_gu(s, pieces):
                if s not in gu_tiles:
                    gu_tiles[s] = gupool.tile([128, KO, I2], BF16, tag="gu",
                                              name=f"gu{s}")
                t = gu_tiles[s]
                for pc in pieces:
                    nc.sync.dma_start(
                        t[:, :, 256 * pc:256 * (pc + 1)],
                        gu_aps[s][:, pc],
                    )

            def load_dn(s):
                t = dnpool.tile([128, JO, H], BF16, tag="dn", name=f"dn{s}")
                nc.sync.dma_start(t[:], dn_aps[s])
                dn_tiles[s] = t

            # head: x(0) in k-halves, first gu piece split gate/up so the
            # very first accumulation run can start as early as possible
            # head FIFO in first-need order: x/gu k-halves for the very
            # first accumulation run, then the rest
            xt0 = xpool.tile([128, KO, 512], BF16, tag="x", name="x0")
            x_tiles[0] = xt0
            gt0 = gupool.tile([128, KO, I2], BF16, tag="gu", name="gu0")
            gu_tiles[0] = gt0
            Ts0 = SEGT[0]
            # interleave x / gu-gate k-quarters so the j0 gate run can
            # begin after ~0.5MB has landed
            for q in range(4):
                ka, kb = 4 * q, 4 * (q + 1)
                nc.sync.dma_start(xt0[:, ka:kb, :Ts0], x_aps[0][:, ka:kb])
                nc.sync.dma_start(gt0[:, ka:kb, 0:256],
                                  gu_aps[0][:, 0, ka:kb, :])
            load_gu(0, (1, 2, 3, 4, 5))
            w_sb = cpool.tile([128, NCH], F32, tag="w")
            nc.sync.dma_start(w_sb[:], w_d.ap())
            load_dn(0)

            actT_tiles = {}

            def m1(s, jrange):
                Ts = SEGT[s]
                xc = x_tiles[s]
                guc = gu_tiles[s]
                if s not in actT_tiles:
                    actT_tiles[s] = apool.tile([128, JO, 512], BF16,
                                               tag="act", name=f"act{s}")
                actT = actT_tiles[s]
                for j in jrange:
                    a_ps = pph.tile([128, 512], F32, tag="hg",
                                    name=f"hg{s}_{j}")
                    b_ps = pph.tile([128, 512], F32, tag="hu",
                                    name=f"hu{s}_{j}")
                    # gate/up interleaved: banks alternate every instruction
                    # so run boundaries and stationary loads stay hidden
                    for k in range(KO):
                        nc.tensor.matmul(
                            a_ps[:, :Ts], guc[:, k, 256 * j:256 * j + 128],
                            xc[:, k, :Ts],
                            start=(k == 0), stop=(k == KO - 1),
                        )
                        nc.tensor.matmul(
                            b_ps[:, :Ts],
                            guc[:, k, 256 * j + 128:256 * j + 256],
                            xc[:, k, :Ts],
                            start=(k == 0), stop=(k == KO - 1),
                        )
                    sl = spool.tile([128, 512], F32, tag="sl",
                                    name=f"sl{s}_{j}")
                    nc.scalar.activation(sl[:, :Ts], a_ps[:, :Ts], ACTF.Silu)
                    nc.vector.tensor_tensor(actT[:, j, :Ts], sl[:, :Ts],
                                            b_ps[:, :Ts], OP.mult)
                    # chain start only: seg1's lookahead inputs must load
                    # during M1'(0); all later prefetches ride M2 windows
                    if s == 0:
                        if j == 0:
                            load_x(1)
                        elif j == 1:
                            load_gu(1, (0, 1, 2))


            def m2(s):
                Ts = SEGT[s]
                x_tiles.pop(s)
                gu_tiles.pop(s)
                actT = actT_tiles.pop(s)
                dnc = dn_tiles.pop(s)
                for sub in range(SEGSUB[s]):
                    np_ = min(128, Ts - 128 * sub)
                    # input prefetch rides M2's otherwise-idle DMA window
                    # instead of contending with M1' SBUF reads
                    if sub == 0 and s + 1 < NSEG:
                        load_gu(s + 1, (3, 4, 5))
                    elif sub == 1 and s + 1 < NSEG:
                        load_dn(s + 1)
                    elif sub == 2 and s + 2 < NSEG:
                        load_x(s + 2)
                        load_gu(s + 2, (0, 1, 2))
                    # two hb blocks share one 2-bank PSUM tile: a single
                    # wide DVE scale + DMA per pair halves drain overhead
                    for hp in range(H // (2 * HBW)):
                        y_ps = ppy.tile([128, 2 * HBW], F32, tag="y_ps",
                                        name=f"y{s}_{sub}_{hp}")
                        # j outer, halves inner: consecutive matmuls share
                        # the stationary and alternate banks
                        for j in range(JO):
                            for hh in range(2):
                                hb = 2 * hp + hh
                                nc.tensor.matmul(
                                    y_ps[:np_, HBW * hh:HBW * (hh + 1)],
                                    actT[:, j, 128 * sub:128 * sub + np_],
                                    dnc[:, j, HBW * hb:HBW * (hb + 1)],
                                    start=(j == 0), stop=(j == JO - 1),
                                )
                        y_sb = ypool.tile([128, 2 * HBW], BF16, tag="y_sb",
                                          name=f"ys{s}_{sub}_{hp}")
                        ci = CHOFF[s] + sub
                        nc.vector.tensor_scalar(y_sb[:np_], y_ps[:np_],
                                                w_sb[:np_, ci:ci + 1], None,
                                                OP.mult)
                        nc.sync.dma_start(
                            y_d.ap()[TOFF[s] + 128 * sub:
                                     TOFF[s] + 128 * sub + np_,
                                     2 * HBW * hp:2 * HBW * (hp + 1)],
                            y_sb[:np_],
                        )

            # 2-stage lookahead: M1'(s+1)'s first two j-pairs are emitted
            # before M2(s), so their DVE mults queue ahead of M2(s)'s
            # y-scales and the seg boundary never stalls on the DVE FIFO.
            m1(0, range(0, 2))
            for s in range(NSEG):
                m1(s, range(2, JO))
                if s + 1 < NSEG:
                    m1(s + 1, range(0, 2))
                m2(s)
    nc.compile()
    _CACHED_NC = nc
    return nc


def _route(x32, gw32, mg):
    """fp32 router matching reference: softmax, top-8, renorm, per-group sum."""
    logits = x32 @ gw32.T
    m = logits.max(axis=-1, keepdims=True)
    p = np.exp(logits - m)
    p /= p.sum(axis=-1, keepdims=True)
    idx = np.argsort(-p, axis=-1, kind="stable")[:, :TOP_K]
    val = np.take_along_axis(p, idx, axis=-1)
    val = val / val.sum(axis=-1, keepdims=True)
    sel = mg[idx]  # [T, K] group ids
    w_tg = np.zeros((x32.shape[0], G), dtype=np.float32)
    np.add.at(w_tg, (np.arange(x32.shape[0])[:, None], sel), val)
    return w_tg


def _solve_parts(counts):
    """Assign each group a multiset of parts from SEGT (8 of each size
    available) covering its token count; all 24 parts are consumed
    (leftovers become all-pad parts on group 0).  Returns per-group
    (n_of_size0, n_of_size1, n_of_size2), minimizing total padding."""
    sizes = SEGT
    states = {(0, 0, 0): (0, ())}
    for c in counts:
        new = {}
        for used, (pad, plan) in states.items():
            for m0 in range(0, 8 - used[0] + 1):
                for m1 in range(0, 8 - used[1] + 1):
                    rem = c - m0 * sizes[0] - m1 * sizes[1]
                    m2min = max(0, -(-rem // sizes[2]))
                    for m2 in (m2min, m2min + 1):
                        if used[2] + m2 > 8:
                            continue
                        cov = m0 * sizes[0] + m1 * sizes[1] + m2 * sizes[2]
                        if cov < c:
                            continue
                        key = (used[0] + m0, used[1] + m1, used[2] + m2)
                        cand = (pad + cov - c, plan + ((m0, m1, m2),))
                        if key not in new or cand[0] < new[key][0]:
                            new[key] = cand
        states = new
    best = None
    for used, (pad, plan) in states.items():
        # leftovers are pure padding
        extra = sum((8 - u) * s for u, s in zip(used, sizes))
        if best is None or pad + extra < best[0]:
            left = tuple(8 - u for u in used)
            best = (pad + extra, plan, left)
    if best is None:
        raise RuntimeError(f"no schedule for token counts {counts}")
    plan = list(best[1])
    plan[0] = tuple(p + l for p, l in zip(plan[0], best[2]))
    return plan


def _schedule(w_tg):
    """Build 8 parts of each SEGT size: (group, token_slots) lists."""
    toks = [np.nonzero(w_tg[:, g] > 0.0)[0] for g in range(G)]
    plan = _solve_parts([len(t) for t in toks])
    parts = {0: [], 1: [], 2: []}
    for g, mult in enumerate(plan):
        cap = sum(m * s for m, s in zip(mult, SEGT))
        slots = np.full(cap, -1, dtype=np.int64)
        slots[:len(toks[g])] = toks[g]
        off = 0
        for si in range(NSEG):
            for _ in range(mult[si]):
                parts[si].append((g, slots[off:off + SEGT[si]]))
                off += SEGT[si]
    return parts


def prepare_in_maps(hidden_states, gate_weight, gate_up_proj, down_proj,
                    merge_groups, dominant_experts):
    x32 = np.asarray(hidden_states, dtype=np.float32).reshape(T, H)
    gw32 = np.asarray(gate_weight, dtype=np.float32)
    mg = np.asarray(merge_groups).astype(np.int64)
    de = np.asarray(dominant_experts).astype(np.int64)
    gup = np.asarray(gate_up_proj, dtype=np.float32)
    dnp_ = np.asarray(down_proj, dtype=np.float32)

    w_tg = _route(x32, gw32, mg)
    parts = _schedule(w_tg)

    # per-expert transformed weights (bf16, partition-major), cached
    guT_c, dnT_c = {}, {}
    for g in range(G):
        e = int(de[g])
        if e not in guT_c:
            guT = gup[e].T[:, _GU_PERM].astype(BF)           # [H, 2I]
            # [p, j, k, c]: element (k*128+p, 256j+c)
            guT_c[e] = np.ascontiguousarray(
                guT.reshape(KO, 128, JO, 256).transpose(1, 2, 0, 3)
            ).reshape(128, -1)
            dnT = dnp_[e].T.astype(BF)                       # [I, H]
            dnT_c[e] = np.ascontiguousarray(
                dnT.reshape(JO, 128, H).transpose(1, 0, 2)
            ).reshape(128, -1)                               # [p, j, h]

    x_bf = x32.astype(BF)
    in_maps = []
    slot_lists = []
    for c in range(G):
        segs = [parts[0][c], parts[1][c], parts[2][c]]
        slots = np.concatenate([sg[1] for sg in segs])       # [NT], -1 pads
        slot_lists.append(slots)
        # w columns follow the (seg, sub) grid of 128-token sub-chunks
        w_dev = np.zeros((128, NCH), dtype=np.float32)
        for s, (g, sl) in enumerate(segs):
            valid = sl >= 0
            wseg = np.zeros(len(sl), dtype=np.float32)
            wseg[valid] = w_tg[sl[valid], g]
            for sub in range(SEGSUB[s]):
                col = wseg[128 * sub:128 * (sub + 1)]
                w_dev[:len(col), CHOFF[s] + sub] = col
        in_map = {"w": np.ascontiguousarray(w_dev)}
        for s, (g, sl) in enumerate(segs):
            cl = np.where(sl < 0, 0, sl)
            xs = x_bf[cl]                                    # [Ts, H]
            # [p, k, t]: element (t, k*128+p)
            in_map[f"x{s}"] = np.ascontiguousarray(
                xs.reshape(len(sl), KO, 128).transpose(2, 1, 0)
            ).reshape(128, -1)
            e = int(de[g])
            in_map[f"gu{s}"] = guT_c[e]
            in_map[f"dn{s}"] = dnT_c[e]
        in_maps.append(in_map)
    return in_maps, slot_lists


def kernel(hidden_states, gate_weight, gate_up_proj, down_proj,
           merge_groups, dominant_experts):
    in_maps, slot_lists = prepare_in_maps(
        hidden_states, gate_weight, gate_up_proj, down_proj,
        merge_groups, dominant_experts)
    nc = _build()
    res = run_bass_kernel_spmd(nc, in_maps, core_ids=list(range(G)),
                               trace=False)
    out = np.zeros((T, H), dtype=np.float64)
    for c, r in enumerate(res.results):
        y = np.asarray(r["y"], dtype=np.float64)             # [NT, H]
        slots = slot_lists[c]
        # segments have unique tokens internally; add per segment
        for so, n in zip(TOFF, SEGT):
            sl = slots[so:so + n]
            valid = sl >= 0
            out[sl[valid]] += y[so:so + n][valid]
    return out.astype(np.float32).reshape(1, T, H)


# revision 61
# speedup vs baseline: 1.0085x; 1.0003x over previous
"""TRN2 Bass kernel for nn_HCSMoEQwen3MoeSparseMoeBlock (8-core).

Host computes the router (fp32 numpy, matching reference softmax/top-8
semantics) and the per-(token, group) combined weight w_tg.  Only
(token, group) pairs with w > 0 are computed on device (~64% density).

Schedule: each group's active tokens are carved into single-group parts
of sizes SEGT = (368, 480, 512) — 8 parts of each size globally, found
by a small DP — so every core runs an identical 3-segment, 1360-token
program (vs 2048 dense).  Per-core inputs: gathered x (bf16,
partition-major so DMA reads are 8-16KB bursts), one gu/dn weight slot
per segment (bf16, gate/up column-interleaved), per-token weights.
Host scatter-adds the bf16 partial outputs (unique tokens per segment).

Device per segment:
  M1 transposed: hT[o-tile, t] = guT-tile-stationary.T @ xT (bf16, fp32
  PSUM, full PE rate, no transposes); gate/up PSUM bank pair -> Silu on
  Scalar * up on DVE -> actT [i, t] bf16.
  M2: y[t, hb] = actT-subtile-stationary.T @ dnT (bf16); two hb blocks
  accumulate into one 2-bank PSUM tile so a single wide DVE scale-by-w
  + one DMA drains the pair (drain keeps up with PE) -> bf16 DRAM.
Software pipelining: M1'(s+1)'s first two j-pairs are emitted before
M2(s) so DVE mults queue ahead of y-scales; gate/up (and M2 half-pair)
matmuls interleave banks every instruction so run boundaries stay
hidden; steady-state input prefetch rides M2's otherwise-idle DMA
windows instead of contending with M1' SBUF reads; head interleaves
x/gu k-quarters so the first matmul starts ~11.5us in (vs ~9us fixed
pre-DMA startup).
"""
import numpy as np
import ml_dtypes

import concourse.bass as bass
import concourse.mybir as mybir
import concourse.tile as tile
from concourse import bacc
from concourse.bass_utils import run_bass_kernel_spmd

BF = ml_dtypes.bfloat16

T = 2048
H = 2048
I2 = 1536
I = 768
E = 32
G = 8
TOP_K = 8
KO = H // 128          # 16 k-tiles
JO = I // 128          # 6 i-tiles
NSEG = 3
# per-core segment token counts (identical on every core; 8 parts of each
# size exist globally, single-group each).  Sizes need not be multiples of
# 128: M1' cost scales with tokens, M2 with ceil(Ts/128), so (368,480,512)
# = 1360 tokens/core beats (384,512,512) = 1408 at equal M2 cost.
SEGT = (368, 480, 512)
SEGSUB = tuple(-(-t // 128) for t in SEGT)   # M2 sub-chunks: (3, 4, 4)
CHOFF = (0, 3, 7)      # w-column offset per segment
TOFF = (0, 368, 848)   # token slot offset per segment
NCH = sum(SEGSUB)      # 11 w columns per core
NT = sum(SEGT)         # 1360 token slots per core
HBW = 512

F32 = mybir.dt.float32
BF16 = mybir.dt.bfloat16
AX = mybir.AxisListType.X
OP = mybir.AluOpType
ACTF = mybir.ActivationFunctionType

_CACHED_NC = None

# gate j-tile at cols [256j, 256j+128), up j-tile at [256j+128, 256j+256)
_GU_PERM = np.concatenate(
    [np.r_[128 * j:128 * j + 128, I + 128 * j:I + 128 * j + 128]
     for j in range(JO)]
)


def _build():
    global _CACHED_NC
    if _CACHED_NC is not None:
        return _CACHED_NC
    nc = bacc.Bacc("TRN2", target_bir_lowering=False, debug=False,
                   num_devices=G)

    # partition-major inputs: per-partition rows are contiguous in DRAM,
    # so DMA reads come in 8-24KB bursts instead of 0.5-1KB lines.
    x_ds = [nc.dram_tensor(f"x{s}", [128, KO * SEGT[s]], BF16,
                           kind="ExternalInput") for s in range(NSEG)]
    gu_ds = [nc.dram_tensor(f"gu{s}", [128, JO * KO * 256], BF16,
                            kind="ExternalInput") for s in range(NSEG)]
    dn_ds = [nc.dram_tensor(f"dn{s}", [128, JO * H], BF16,
                            kind="ExternalInput") for s in range(NSEG)]
    w_d = nc.dram_tensor("w", [128, NCH], F32, kind="ExternalInput")
    y_d = nc.dram_tensor("y", [NT, H], BF16, kind="ExternalOutput")

    x_aps = [x_ds[s].ap().rearrange("p (k t) -> p k t", k=KO)
             for s in range(NSEG)]
    gu_aps = [gu_ds[s].ap().rearrange("p (j k c) -> p j k c", j=JO, k=KO)
              for s in range(NSEG)]
    dn_aps = [dn_ds[s].ap().rearrange("p (j h) -> p j h", j=JO)
              for s in range(NSEG)]

    with tile.TileContext(nc) as tc:
        with (
            tc.tile_pool(name="const", bufs=1) as cpool,
            tc.tile_pool(name="guw", bufs=2) as gupool,
            tc.tile_pool(name="dnw", bufs=2) as dnpool,
            tc.tile_pool(name="xin", bufs=2) as xpool,
            tc.tile_pool(name="acts", bufs=2) as apool,
            tc.tile_pool(name="silu", bufs=2) as spool,
            tc.tile_pool(name="yout", bufs=4) as ypool,
            tc.tile_pool(name="ph", bufs=2, space="PSUM") as pph,
            tc.tile_pool(name="py", bufs=2, space="PSUM") as ppy,
        ):
            x_tiles = {}
            gu_tiles = {}
            dn_tiles = {}

            def load_x(s, halves=False):
                t = xpool.tile([128, KO, 512], BF16, tag="x", name=f"x{s}")
                if halves:
                    hk = KO // 2
                    nc.sync.dma_start(t[:, :hk, :SEGT[s]], x_aps[s][:, :hk])
                    nc.sync.dma_start(t[:, hk:, :SEGT[s]], x_aps[s][:, hk:])
                else:
                    nc.sync.dma_start(t[:, :, :SEGT[s]], x_aps[s])
                x_tiles[s] = t

            def load_gu(s, pieces):
                if s not in gu_tiles:
                    gu_tiles[s] = gupool.tile([128, KO, I2], BF16, tag="gu",
                                              name=f"gu{s}")
                t = gu_tiles[s]
                for pc in pieces:
                    nc.sync.dma_start(
                        t[:, :, 256 * pc:256 * (pc + 1)],
                        gu_aps[s][:, pc],
                    )

            def load_dn(s):
                t = dnpool.tile([128, JO, H], BF16, tag="dn", name=f"dn{s}")
                nc.sync.dma_start(t[:], dn_aps[s])
                dn_tiles[s] = t

            # head: x(0) in k-halves, first gu piece split gate/up so the
            # very first accumulation run can start as early as possible
            # head FIFO in first-need order: x/gu k-halves for the very
            # first accumulation run, then the rest
            xt0 = xpool.tile([128, KO, 512], BF16, tag="x", name="x0")
            x_tiles[0] = xt0
            gt0 = gupool.tile([128, KO, I2], BF16, tag="gu", name="gu0")
            gu_tiles[0] = gt0
            Ts0 = SEGT[0]
            # interleave x / gu-gate k-quarters so the j0 gate run can
            # begin after ~0.5MB has landed
            for q in range(4):
                ka, kb = 4 * q, 4 * (q + 1)
                nc.sync.dma_start(xt0[:, ka:kb, :Ts0], x_aps[0][:, ka:kb])
                nc.sync.dma_start(gt0[:, ka:kb, 0:256],
                                  gu_aps[0][:, 0, ka:kb, :])
            load_gu(0, (1, 2, 3, 4, 5))
            w_sb = cpool.tile([128, NCH], F32, tag="w")
            nc.sync.dma_start(w_sb[:], w_d.ap())
            load_dn(0)

            actT_tiles = {}

            def m1(s, jrange):
                Ts = SEGT[s]
                xc = x_tiles[s]
                guc = gu_tiles[s]
                if s not in actT_tiles:
                    actT_tiles[s] = apool.tile([128, JO, 512], BF16,
                                               tag="act", name=f"act{s}")
                actT = actT_tiles[s]
                for j in jrange:
                    a_ps = pph.tile([128, 512], F32, tag="hg",
                                    name=f"hg{s}_{j}")
                    b_ps = pph.tile([128, 512], F32, tag="hu",
                                    name=f"hu{s}_{j}")
                    # gate/up interleaved: banks alternate every instruction
                    # so run boundaries and stationary loads stay hidden
                    for k in range(KO):
                        nc.tensor.matmul(
                            a_ps[:, :Ts], guc[:, k, 256 * j:256 * j + 128],
                            xc[:, k, :Ts],
                            start=(k == 0), stop=(k == KO - 1),
                        )
                        nc.tensor.matmul(
                            b_ps[:, :Ts],
                            guc[:, k, 256 * j + 128:256 * j + 256],
                            xc[:, k, :Ts],
                            start=(k == 0), stop=(k == KO - 1),
                        )
                    sl = spool.tile([128, 512], F32, tag="sl",
                                    name=f"sl{s}_{j}")
                    nc.scalar.activation(sl[:, :Ts], a_ps[:, :Ts], ACTF.Silu)
                    nc.vector.tensor_tensor(actT[:, j, :Ts], sl[:, :Ts],
                                            b_ps[:, :Ts], OP.mult)
                    # chain start only: seg1's lookahead inputs must load
                    # during M1'(0); all later prefetches ride M2 windows
                    if s == 0:
                        if j == 0:
                            load_x(1)
                        elif j == 1:
                            load_gu(1, (0, 1, 2))


            def m2(s):
                Ts = SEGT[s]
                x_tiles.pop(s)
                gu_tiles.pop(s)
                actT = actT_tiles.pop(s)
                dnc = dn_tiles.pop(s)
                for sub in range(SEGSUB[s]):
                    np_ = min(128, Ts - 128 * sub)
                    # input prefetch rides M2's otherwise-idle DMA window
                    # instead of contending with M1' SBUF reads
                    if sub == 0 and s + 1 < NSEG:
                        load_gu(s + 1, (3, 4, 5))
                    elif sub == 1 and s + 1 < NSEG:
                        load_dn(s + 1)
                    elif sub == 2 and s + 2 < NSEG:
                        load_x(s + 2)
                        load_gu(s + 2, (0, 1, 2))
                    # two hb blocks share one 2-bank PSUM tile: a single
                    # wide DVE scale + DMA per pair halves drain overhead.
                    # The very last unit is emitted as two 512 singles so
                    # the first half's drain hides under the second half's
                    # matmuls, shortening the exposed tail.
                    last_sub = (s == NSEG - 1 and sub == SEGSUB[s] - 1)
                    ci = CHOFF[s] + sub
                    row0 = TOFF[s] + 128 * sub
                    for hp in range(H // (2 * HBW)):
                        if last_sub and hp == 1:
                            for hh in range(2):
                                hb = 2 * hp + hh
                                y_ps = ppy.tile([128, HBW], F32,
                                                tag="y_ps",
                                                name=f"yf{hh}")
                                for j in range(JO):
                                    nc.tensor.matmul(
                                        y_ps[:np_],
                                        actT[:, j,
                                             128 * sub:128 * sub + np_],
                                        dnc[:, j, HBW * hb:HBW * (hb + 1)],
                                        start=(j == 0), stop=(j == JO - 1),
                                    )
                                y_sb = ypool.tile([128, HBW], BF16,
                                                  tag="y_sb",
                                                  name=f"ysf{hh}")
                                nc.vector.tensor_scalar(
                                    y_sb[:np_], y_ps[:np_],
                                    w_sb[:np_, ci:ci + 1], None, OP.mult)
                                nc.sync.dma_start(
                                    y_d.ap()[row0:row0 + np_,
                                             HBW * hb:HBW * (hb + 1)],
                                    y_sb[:np_],
                                )
                            continue
                        y_ps = ppy.tile([128, 2 * HBW], F32, tag="y_ps",
                                        name=f"y{s}_{sub}_{hp}")
                        # j outer, halves inner: consecutive matmuls share
                        # the stationary and alternate banks
                        for j in range(JO):
                            for hh in range(2):
                                hb = 2 * hp + hh
                                nc.tensor.matmul(
                                    y_ps[:np_, HBW * hh:HBW * (hh + 1)],
                                    actT[:, j, 128 * sub:128 * sub + np_],
                                    dnc[:, j, HBW * hb:HBW * (hb + 1)],
                                    start=(j == 0), stop=(j == JO - 1),
                                )
                        y_sb = ypool.tile([128, 2 * HBW], BF16, tag="y_sb",
                                          name=f"ys{s}_{sub}_{hp}")
                        nc.vector.tensor_scalar(y_sb[:np_], y_ps[:np_],
                                                w_sb[:np_, ci:ci + 1], None,
                                                OP.mult)
                        nc.sync.dma_start(
                            y_d.ap()[row0:row0 + np_,
                                     2 * HBW * hp:2 * HBW * (hp + 1)],
                            y_sb[:np_],
                        )

            # 2-stage lookahead: M1'(s+1)'s first two j-pairs are emitted
            # before M2(s), so their DVE mults queue ahead of M2(s)'s
            # y-scales and the seg boundary never stalls on the DVE FIFO.
            m1(0, range(0, 2))
            for s in range(NSEG):
                m1(s, range(2, JO))
                if s + 1 < NSEG:
                    m1(s + 1, range(0, 2))
                m2(s)
    nc.compile()
    _CACHED_NC = nc
    return nc


def _route(x32, gw32, mg):
    """fp32 router matching reference: softmax, top-8, renorm, per-group sum."""
    logits = x32 @ gw32.T
    m = logits.max(axis=-1, keepdims=True)
    p = np.exp(logits - m)
    p /= p.sum(axis=-1, keepdims=True)
    idx = np.argsort(-p, axis=-1, kind="stable")[:, :TOP_K]
    val = np.take_along_axis(p, idx, axis=-1)
    val = val / val.sum(axis=-1, keepdims=True)
    sel = mg[idx]  # [T, K] group ids
    w_tg = np.zeros((x32.shape[0], G), dtype=np.float32)
    np.add.at(w_tg, (np.arange(x32.shape[0])[:, None], sel), val)
    return w_tg


def _solve_parts(counts):
    """Assign each group a multiset of parts from SEGT (8 of each size
    available) covering its token count; all 24 parts are consumed
    (leftovers become all-pad parts on group 0).  Returns per-group
    (n_of_size0, n_of_size1, n_of_size2), minimizing total padding."""
    sizes = SEGT
    states = {(0, 0, 0): (0, ())}
    for c in counts:
        new = {}
        for used, (pad, plan) in states.items():
            for m0 in range(0, 8 - used[0] + 1):
                for m1 in range(0, 8 - used[1] + 1):
                    rem = c - m0 * sizes[0] - m1 * sizes[1]
                    m2min = max(0, -(-rem // sizes[2]))
                    for m2 in (m2min, m2min + 1):
                        if used[2] + m2 > 8:
                            continue
                        cov = m0 * sizes[0] + m1 * sizes[1] + m2 * sizes[2]
                        if cov < c:
                            continue
                        key = (used[0] + m0, used[1] + m1, used[2] + m2)
                        cand = (pad + cov - c, plan + ((m0, m1, m2),))
                        if key not in new or cand[0] < new[key][0]:
                            new[key] = cand
        states = new
    best = None
    for used, (pad, plan) in states.items():
        # leftovers are pure padding
        extra = sum((8 - u) * s for u, s in zip(used, sizes))
        if best is None or pad + extra < best[0]:
            left = tuple(8 - u for u in used)
            best = (pad + extra, plan, left)
    if best is None:
        raise RuntimeError(f"no schedule for token counts {counts}")
    plan = list(best[1])
    plan[0] = tuple(p + l for p, l in zip(plan[0], best[2]))
    return plan


def _schedule(w_tg):
    """Build 8 parts of each SEGT size: (group, token_slots) lists."""
    toks = [np.nonzero(w_tg[:, g] > 0.0)[0] for g in range(G)]
    plan = _solve_parts([len(t) for t in toks])
    parts = {0: [], 1: [], 2: []}
    for g, mult in enumerate(plan):
        cap = sum(m * s for m, s in zip(mult, SEGT))
        slots = np.full(cap, -1, dtype=np.int64)
        slots[:len(toks[g])] = toks[g]
        off = 0
        for si in range(NSEG):
            for _ in range(mult[si]):
                parts[si].append((g, slots[off:off + SEGT[si]]))
                off += SEGT[si]
    return parts


def prepare_in_maps(hidden_states, gate_weight, gate_up_proj, down_proj,
                    merge_groups, dominant_experts):
    x32 = np.asarray(hidden_states, dtype=np.float32).reshape(T, H)
    gw32 = np.asarray(gate_weight, dtype=np.float32)
    mg = np.asarray(merge_groups).astype(np.int64)
    de = np.asarray(dominant_experts).astype(np.int64)
    gup = np.asarray(gate_up_proj, dtype=np.float32)
    dnp_ = np.asarray(down_proj, dtype=np.float32)

    w_tg = _route(x32, gw32, mg)
    parts = _schedule(w_tg)

    # per-expert transformed weights (bf16, partition-major), cached
    guT_c, dnT_c = {}, {}
    for g in range(G):
        e = int(de[g])
        if e not in guT_c:
            guT = gup[e].T[:, _GU_PERM].astype(BF)           # [H, 2I]
            # [p, j, k, c]: element (k*128+p, 256j+c)
            guT_c[e] = np.ascontiguousarray(
                guT.reshape(KO, 128, JO, 256).transpose(1, 2, 0, 3)
            ).reshape(128, -1)
            dnT = dnp_[e].T.astype(BF)                       # [I, H]
            dnT_c[e] = np.ascontiguousarray(
                dnT.reshape(JO, 128, H).transpose(1, 0, 2)
            ).reshape(128, -1)                               # [p, j, h]

    x_bf = x32.astype(BF)
    in_maps = []
    slot_lists = []
    for c in range(G):
        segs = [parts[0][c], parts[1][c], parts[2][c]]
        slots = np.concatenate([sg[1] for sg in segs])       # [NT], -1 pads
        slot_lists.append(slots)
        # w columns follow the (seg, sub) grid of 128-token sub-chunks
        w_dev = np.zeros((128, NCH), dtype=np.float32)
        for s, (g, sl) in enumerate(segs):
            valid = sl >= 0
            wseg = np.zeros(len(sl), dtype=np.float32)
            wseg[valid] = w_tg[sl[valid], g]
            for sub in range(SEGSUB[s]):
                col = wseg[128 * sub:128 * (sub + 1)]
                w_dev[:len(col), CHOFF[s] + sub] = col
        in_map = {"w": np.ascontiguousarray(w_dev)}
        for s, (g, sl) in enumerate(segs):
            cl = np.where(sl < 0, 0, sl)
            xs = x_bf[cl]                                    # [Ts, H]
            # [p, k, t]: element (t, k*128+p)
            in_map[f"x{s}"] = np.ascontiguousarray(
                xs.reshape(len(sl), KO, 128).transpose(2, 1, 0)
            ).reshape(128, -1)
            e = int(de[g])
            in_map[f"gu{s}"] = guT_c[e]
            in_map[f"dn{s}"] = dnT_c[e]
        in_maps.append(in_map)
    return in_maps, slot_lists


def kernel(hidden_states, gate_weight, gate_up_proj, down_proj,
           merge_groups, dominant_experts):
    in_maps, slot_lists = prepare_in_maps(
        hidden_states, gate_weight, gate_up_proj, down_proj,
        merge_groups, dominant_experts)
    nc = _build()
    res = run_bass_kernel_spmd(nc, in_maps, core_ids=list(range(G)),
                               trace=False)
    out = np.zeros((T, H), dtype=np.float64)
    for c, r in enumerate(res.results):
        y = np.asarray(r["y"], dtype=np.float64)             # [NT, H]
        slots = slot_lists[c]
        # segments have unique tokens internally; add per segment
        for so, n in zip(TOFF, SEGT):
            sl = slots[so:so + n]
            valid = sl >= 0
            out[sl[valid]] += y[so:so + n][valid]
    return out.astype(np.float32).reshape(1, T, H)


# revision 62
# speedup vs baseline: 1.0187x; 1.0100x over previous
"""TRN2 Bass kernel for nn_HCSMoEQwen3MoeSparseMoeBlock (8-core).

Host computes the router (fp32 numpy, matching reference softmax/top-8
semantics) and the per-(token, group) combined weight w_tg.  Only
(token, group) pairs with w > 0 are computed on device (~64% density).

Schedule: each group's active tokens are carved into single-group parts
of sizes SEGT = (368, 480, 512) — 8 parts of each size globally, found
by a small DP — so every core runs an identical 3-segment, 1360-token
program (vs 2048 dense).  Per-core inputs: gathered x (bf16,
partition-major so DMA reads are 8-16KB bursts), one gu/dn weight slot
per segment (bf16, gate/up column-interleaved), per-token weights.
Host scatter-adds the bf16 partial outputs (unique tokens per segment).

Device per segment:
  M1 transposed: hT[o-tile, t] = guT-tile-stationary.T @ xT (bf16, fp32
  PSUM, full PE rate, no transposes); gate/up PSUM bank pair -> Silu on
  Scalar * up on DVE -> actT [i, t] bf16.
  M2: y[t, hb] = actT-subtile-stationary.T @ dnT (bf16); two hb blocks
  accumulate into one 2-bank PSUM tile so a single wide DVE scale-by-w
  + one DMA drains the pair (drain keeps up with PE) -> bf16 DRAM.
Software pipelining: M1'(s+1)'s first two j-pairs are emitted before
M2(s) so DVE mults queue ahead of y-scales; gate/up (and M2 half-pair)
matmuls interleave banks every instruction so run boundaries stay
hidden; steady-state input prefetch rides M2's otherwise-idle DMA
windows instead of contending with M1' SBUF reads; head interleaves
x/gu k-quarters so the first matmul starts ~11.5us in (vs ~9us fixed
pre-DMA startup).
"""
import numpy as np
import ml_dtypes

import concourse.bass as bass
import concourse.mybir as mybir
import concourse.tile as tile
from concourse import bacc
from concourse.bass_utils import run_bass_kernel_spmd

BF = ml_dtypes.bfloat16

T = 2048
H = 2048
I2 = 1536
I = 768
E = 32
G = 8
TOP_K = 8
KO = H // 128          # 16 k-tiles
JO = I // 128          # 6 i-tiles
NSEG = 3
# per-core segment token counts (identical on every core; 8 parts of each
# size exist globally, single-group each).  Sizes need not be multiples of
# 128: M1' cost scales with tokens, M2 with ceil(Ts/128), so (368,480,512)
# = 1360 tokens/core beats (384,512,512) = 1408 at equal M2 cost.
SEGT = (368, 480, 512)
SEGSUB = tuple(-(-t // 128) for t in SEGT)   # M2 sub-chunks: (3, 4, 4)
CHOFF = (0, 3, 7)      # w-column offset per segment
TOFF = (0, 368, 848)   # token slot offset per segment
NCH = sum(SEGSUB)      # 11 w columns per core
NT = sum(SEGT)         # 1360 token slots per core
HBW = 512

F32 = mybir.dt.float32
BF16 = mybir.dt.bfloat16
AX = mybir.AxisListType.X
OP = mybir.AluOpType
ACTF = mybir.ActivationFunctionType

_CACHED_NC = None

# gate j-tile at cols [256j, 256j+128), up j-tile at [256j+128, 256j+256)
_GU_PERM = np.concatenate(
    [np.r_[128 * j:128 * j + 128, I + 128 * j:I + 128 * j + 128]
     for j in range(JO)]
)


def _build():
    global _CACHED_NC
    if _CACHED_NC is not None:
        return _CACHED_NC
    nc = bacc.Bacc("TRN2", target_bir_lowering=False, debug=False,
                   num_devices=G)

    # partition-major inputs: per-partition rows are contiguous in DRAM,
    # so DMA reads come in 8-24KB bursts instead of 0.5-1KB lines.
    x_ds = [nc.dram_tensor(f"x{s}", [128, KO * SEGT[s]], BF16,
                           kind="ExternalInput") for s in range(NSEG)]
    gu_ds = [nc.dram_tensor(f"gu{s}", [128, JO * KO * 256], BF16,
                            kind="ExternalInput") for s in range(NSEG)]
    dn_ds = [nc.dram_tensor(f"dn{s}", [128, JO * H], BF16,
                            kind="ExternalInput") for s in range(NSEG)]
    w_d = nc.dram_tensor("w", [128, NCH], F32, kind="ExternalInput")
    y_d = nc.dram_tensor("y", [NT, H], BF16, kind="ExternalOutput")

    x_aps = [x_ds[s].ap().rearrange("p (k t) -> p k t", k=KO)
             for s in range(NSEG)]
    gu_aps = [gu_ds[s].ap().rearrange("p (j k c) -> p j k c", j=JO, k=KO)
              for s in range(NSEG)]
    dn_aps = [dn_ds[s].ap().rearrange("p (j h) -> p j h", j=JO)
              for s in range(NSEG)]

    with tile.TileContext(nc) as tc:
        with (
            tc.tile_pool(name="const", bufs=1) as cpool,
            tc.tile_pool(name="guw", bufs=2) as gupool,
            tc.tile_pool(name="dnw", bufs=2) as dnpool,
            tc.tile_pool(name="xin", bufs=2) as xpool,
            tc.tile_pool(name="acts", bufs=2) as apool,
            tc.tile_pool(name="silu", bufs=2) as spool,
            tc.tile_pool(name="yout", bufs=4) as ypool,
            tc.tile_pool(name="ph", bufs=2, space="PSUM") as pph,
            tc.tile_pool(name="py", bufs=2, space="PSUM") as ppy,
        ):
            x_tiles = {}
            gu_tiles = {}
            dn_tiles = {}

            def load_x(s, halves=False):
                t = xpool.tile([128, KO, 512], BF16, tag="x", name=f"x{s}")
                if halves:
                    hk = KO // 2
                    nc.sync.dma_start(t[:, :hk, :SEGT[s]], x_aps[s][:, :hk])
                    nc.sync.dma_start(t[:, hk:, :SEGT[s]], x_aps[s][:, hk:])
                else:
                    nc.sync.dma_start(t[:, :, :SEGT[s]], x_aps[s])
                x_tiles[s] = t

            def load_gu(s, pieces):
                if s not in gu_tiles:
                    gu_tiles[s] = gupool.tile([128, KO, I2], BF16, tag="gu",
                                              name=f"gu{s}")
                t = gu_tiles[s]
                for pc in pieces:
                    nc.sync.dma_start(
                        t[:, :, 256 * pc:256 * (pc + 1)],
                        gu_aps[s][:, pc],
                    )

            def load_dn(s):
                t = dnpool.tile([128, JO, H], BF16, tag="dn", name=f"dn{s}")
                nc.sync.dma_start(t[:], dn_aps[s])
                dn_tiles[s] = t

            # head: x(0) in k-halves, first gu piece split gate/up so the
            # very first accumulation run can start as early as possible
            # head FIFO in first-need order: x/gu k-halves for the very
            # first accumulation run, then the rest
            xt0 = xpool.tile([128, KO, 512], BF16, tag="x", name="x0")
            x_tiles[0] = xt0
            gt0 = gupool.tile([128, KO, I2], BF16, tag="gu", name="gu0")
            gu_tiles[0] = gt0
            Ts0 = SEGT[0]
            # interleave x / gu-gate k-quarters so the j0 gate run can
            # begin after ~0.5MB has landed
            for q in range(4):
                ka, kb = 4 * q, 4 * (q + 1)
                nc.sync.dma_start(xt0[:, ka:kb, :Ts0], x_aps[0][:, ka:kb])
                nc.sync.dma_start(gt0[:, ka:kb, 0:256],
                                  gu_aps[0][:, 0, ka:kb, :])
            load_gu(0, (1, 2, 3, 4, 5))
            w_sb = cpool.tile([128, NCH], F32, tag="w")
            nc.sync.dma_start(w_sb[:], w_d.ap())
            load_dn(0)

            actT_tiles = {}

            def m1(s, jrange):
                Ts = SEGT[s]
                xc = x_tiles[s]
                guc = gu_tiles[s]
                if s not in actT_tiles:
                    actT_tiles[s] = apool.tile([128, JO, 512], BF16,
                                               tag="act", name=f"act{s}")
                actT = actT_tiles[s]
                for j in jrange:
                    a_ps = pph.tile([128, 512], F32, tag="hg",
                                    name=f"hg{s}_{j}")
                    b_ps = pph.tile([128, 512], F32, tag="hu",
                                    name=f"hu{s}_{j}")
                    # gate/up interleaved: banks alternate every instruction
                    # so run boundaries and stationary loads stay hidden
                    for k in range(KO):
                        nc.tensor.matmul(
                            a_ps[:, :Ts], guc[:, k, 256 * j:256 * j + 128],
                            xc[:, k, :Ts],
                            start=(k == 0), stop=(k == KO - 1),
                        )
                        nc.tensor.matmul(
                            b_ps[:, :Ts],
                            guc[:, k, 256 * j + 128:256 * j + 256],
                            xc[:, k, :Ts],
                            start=(k == 0), stop=(k == KO - 1),
                        )
                    # stage the gate bank through DVE so the Scalar engine
                    # never reads PSUM (Scalar PSUM reads stall PE matmuls)
                    cp = spool.tile([128, 512], F32, tag="acp",
                                    name=f"cp{s}_{j}")
                    nc.vector.tensor_copy(cp[:, :Ts], a_ps[:, :Ts])
                    sl = spool.tile([128, 512], F32, tag="sl",
                                    name=f"sl{s}_{j}")
                    nc.scalar.activation(sl[:, :Ts], cp[:, :Ts], ACTF.Silu)
                    nc.vector.tensor_tensor(actT[:, j, :Ts], sl[:, :Ts],
                                            b_ps[:, :Ts], OP.mult)
                    # chain start only: seg1's lookahead inputs must load
                    # during M1'(0); all later prefetches ride M2 windows
                    if s == 0:
                        if j == 0:
                            load_x(1)
                        elif j == 1:
                            load_gu(1, (0, 1, 2))


            def m2(s):
                Ts = SEGT[s]
                x_tiles.pop(s)
                gu_tiles.pop(s)
                actT = actT_tiles.pop(s)
                dnc = dn_tiles.pop(s)
                for sub in range(SEGSUB[s]):
                    np_ = min(128, Ts - 128 * sub)
                    # input prefetch rides M2's otherwise-idle DMA window
                    # instead of contending with M1' SBUF reads
                    if sub == 0 and s + 1 < NSEG:
                        load_gu(s + 1, (3, 4, 5))
                    elif sub == 1 and s + 1 < NSEG:
                        load_dn(s + 1)
                    elif sub == 2 and s + 2 < NSEG:
                        load_x(s + 2)
                        load_gu(s + 2, (0, 1, 2))
                    # two hb blocks share one 2-bank PSUM tile: a single
                    # wide DVE scale + DMA per pair halves drain overhead.
                    # The very last unit is emitted as two 512 singles so
                    # the first half's drain hides under the second half's
                    # matmuls, shortening the exposed tail.
                    last_sub = (s == NSEG - 1 and sub == SEGSUB[s] - 1)
                    ci = CHOFF[s] + sub
                    row0 = TOFF[s] + 128 * sub
                    for hp in range(H // (2 * HBW)):
                        if last_sub and hp == 1:
                            for hh in range(2):
                                hb = 2 * hp + hh
                                y_ps = ppy.tile([128, HBW], F32,
                                                tag="y_ps",
                                                name=f"yf{hh}")
                                for j in range(JO):
                                    nc.tensor.matmul(
                                        y_ps[:np_],
                                        actT[:, j,
                                             128 * sub:128 * sub + np_],
                                        dnc[:, j, HBW * hb:HBW * (hb + 1)],
                                        start=(j == 0), stop=(j == JO - 1),
                                    )
                                y_sb = ypool.tile([128, HBW], BF16,
                                                  tag="y_sb",
                                                  name=f"ysf{hh}")
                                nc.vector.tensor_scalar(
                                    y_sb[:np_], y_ps[:np_],
                                    w_sb[:np_, ci:ci + 1], None, OP.mult)
                                nc.sync.dma_start(
                                    y_d.ap()[row0:row0 + np_,
                                             HBW * hb:HBW * (hb + 1)],
                                    y_sb[:np_],
                                )
                            continue
                        y_ps = ppy.tile([128, 2 * HBW], F32, tag="y_ps",
                                        name=f"y{s}_{sub}_{hp}")
                        # j outer, halves inner: consecutive matmuls share
                        # the stationary and alternate banks
                        for j in range(JO):
                            for hh in range(2):
                                hb = 2 * hp + hh
                                nc.tensor.matmul(
                                    y_ps[:np_, HBW * hh:HBW * (hh + 1)],
                                    actT[:, j, 128 * sub:128 * sub + np_],
                                    dnc[:, j, HBW * hb:HBW * (hb + 1)],
                                    start=(j == 0), stop=(j == JO - 1),
                                )
                        y_sb = ypool.tile([128, 2 * HBW], BF16, tag="y_sb",
                                          name=f"ys{s}_{sub}_{hp}")
                        nc.vector.tensor_scalar(y_sb[:np_], y_ps[:np_],
                                                w_sb[:np_, ci:ci + 1], None,
                                                OP.mult)
                        nc.sync.dma_start(
                            y_d.ap()[row0:row0 + np_,
                                     2 * HBW * hp:2 * HBW * (hp + 1)],
                            y_sb[:np_],
                        )

            # 2-stage lookahead: M1'(s+1)'s first two j-pairs are emitted
            # before M2(s), so their DVE mults queue ahead of M2(s)'s
            # y-scales and the seg boundary never stalls on the DVE FIFO.
            m1(0, range(0, 2))
            for s in range(NSEG):
                m1(s, range(2, JO))
                if s + 1 < NSEG:
                    m1(s + 1, range(0, 2))
                m2(s)
    nc.compile()
    _CACHED_NC = nc
    return nc


def _route(x32, gw32, mg):
    """fp32 router matching reference: softmax, top-8, renorm, per-group sum."""
    logits = x32 @ gw32.T
    m = logits.max(axis=-1, keepdims=True)
    p = np.exp(logits - m)
    p /= p.sum(axis=-1, keepdims=True)
    idx = np.argsort(-p, axis=-1, kind="stable")[:, :TOP_K]
    val = np.take_along_axis(p, idx, axis=-1)
    val = val / val.sum(axis=-1, keepdims=True)
    sel = mg[idx]  # [T, K] group ids
    w_tg = np.zeros((x32.shape[0], G), dtype=np.float32)
    np.add.at(w_tg, (np.arange(x32.shape[0])[:, None], sel), val)
    return w_tg


def _solve_parts(counts):
    """Assign each group a multiset of parts from SEGT (8 of each size
    available) covering its token count; all 24 parts are consumed
    (leftovers become all-pad parts on group 0).  Returns per-group
    (n_of_size0, n_of_size1, n_of_size2), minimizing total padding."""
    sizes = SEGT
    states = {(0, 0, 0): (0, ())}
    for c in counts:
        new = {}
        for used, (pad, plan) in states.items():
            for m0 in range(0, 8 - used[0] + 1):
                for m1 in range(0, 8 - used[1] + 1):
                    rem = c - m0 * sizes[0] - m1 * sizes[1]
                    m2min = max(0, -(-rem // sizes[2]))
                    for m2 in (m2min, m2min + 1):
                        if used[2] + m2 > 8:
                            continue
                        cov = m0 * sizes[0] + m1 * sizes[1] + m2 * sizes[2]
                        if cov < c:
                            continue
                        key = (used[0] + m0, used[1] + m1, used[2] + m2)
                        cand = (pad + cov - c, plan + ((m0, m1, m2),))
                        if key not in new or cand[0] < new[key][0]:
                            new[key] = cand
        states = new
    best = None
    for used, (pad, plan) in states.items():
        # leftovers are pure padding
        extra = sum((8 - u) * s for u, s in zip(used, sizes))
        if best is None or pad + extra < best[0]:
            left = tuple(8 - u for u in used)
            best = (pad + extra, plan, left)
    if best is None:
        raise RuntimeError(f"no schedule for token counts {counts}")
    plan = list(best[1])
    plan[0] = tuple(p + l for p, l in zip(plan[0], best[2]))
    return plan


def _schedule(w_tg):
    """Build 8 parts of each SEGT size: (group, token_slots) lists."""
    toks = [np.nonzero(w_tg[:, g] > 0.0)[0] for g in range(G)]
    plan = _solve_parts([len(t) for t in toks])
    parts = {0: [], 1: [], 2: []}
    for g, mult in enumerate(plan):
        cap = sum(m * s for m, s in zip(mult, SEGT))
        slots = np.full(cap, -1, dtype=np.int64)
        slots[:len(toks[g])] = toks[g]
        off = 0
        for si in range(NSEG):
            for _ in range(mult[si]):
                parts[si].append((g, slots[off:off + SEGT[si]]))
                off += SEGT[si]
    return parts


def prepare_in_maps(hidden_states, gate_weight, gate_up_proj, down_proj,
                    merge_groups, dominant_experts):
    x32 = np.asarray(hidden_states, dtype=np.float32).reshape(T, H)
    gw32 = np.asarray(gate_weight, dtype=np.float32)
    mg = np.asarray(merge_groups).astype(np.int64)
    de = np.asarray(dominant_experts).astype(np.int64)
    gup = np.asarray(gate_up_proj, dtype=np.float32)
    dnp_ = np.asarray(down_proj, dtype=np.float32)

    w_tg = _route(x32, gw32, mg)
    parts = _schedule(w_tg)

    # per-expert transformed weights (bf16, partition-major), cached
    guT_c, dnT_c = {}, {}
    for g in range(G):
        e = int(de[g])
        if e not in guT_c:
            guT = gup[e].T[:, _GU_PERM].astype(BF)           # [H, 2I]
            # [p, j, k, c]: element (k*128+p, 256j+c)
            guT_c[e] = np.ascontiguousarray(
                guT.reshape(KO, 128, JO, 256).transpose(1, 2, 0, 3)
            ).reshape(128, -1)
            dnT = dnp_[e].T.astype(BF)                       # [I, H]
            dnT_c[e] = np.ascontiguousarray(
                dnT.reshape(JO, 128, H).transpose(1, 0, 2)
            ).reshape(128, -1)                               # [p, j, h]

    x_bf = x32.astype(BF)
    in_maps = []
    slot_lists = []
    for c in range(G):
        segs = [parts[0][c], parts[1][c], parts[2][c]]
        slots = np.concatenate([sg[1] for sg in segs])       # [NT], -1 pads
        slot_lists.append(slots)
        # w columns follow the (seg, sub) grid of 128-token sub-chunks
        w_dev = np.zeros((128, NCH), dtype=np.float32)
        for s, (g, sl) in enumerate(segs):
            valid = sl >= 0
            wseg = np.zeros(len(sl), dtype=np.float32)
            wseg[valid] = w_tg[sl[valid], g]
            for sub in range(SEGSUB[s]):
                col = wseg[128 * sub:128 * (sub + 1)]
                w_dev[:len(col), CHOFF[s] + sub] = col
        in_map = {"w": np.ascontiguousarray(w_dev)}
        for s, (g, sl) in enumerate(segs):
            cl = np.where(sl < 0, 0, sl)
            xs = x_bf[cl]                                    # [Ts, H]
            # [p, k, t]: element (t, k*128+p)
            in_map[f"x{s}"] = np.ascontiguousarray(
                xs.reshape(len(sl), KO, 128).transpose(2, 1, 0)
            ).reshape(128, -1)
            e = int(de[g])
            in_map[f"gu{s}"] = guT_c[e]
            in_map[f"dn{s}"] = dnT_c[e]
        in_maps.append(in_map)
    return in_maps, slot_lists


def kernel(hidden_states, gate_weight, gate_up_proj, down_proj,
           merge_groups, dominant_experts):
    in_maps, slot_lists = prepare_in_maps(
        hidden_states, gate_weight, gate_up_proj, down_proj,
        merge_groups, dominant_experts)
    nc = _build()
    res = run_bass_kernel_spmd(nc, in_maps, core_ids=list(range(G)),
                               trace=False)
    out = np.zeros((T, H), dtype=np.float64)
    for c, r in enumerate(res.results):
        y = np.asarray(r["y"], dtype=np.float64)             # [NT, H]
        slots = slot_lists[c]
        # segments have unique tokens internally; add per segment
        for so, n in zip(TOFF, SEGT):
            sl = slots[so:so + n]
            valid = sl >= 0
            out[sl[valid]] += y[so:so + n][valid]
    return out.astype(np.float32).reshape(1, T, H)
